# revision 18
# baseline (speedup 1.0000x reference)
"""MixedDecoder (dense MoE blend) Trainium2 kernel, v5.

Data-parallel over 8 NeuronCores: batch 512 -> 64 rows/core, expert weights
replicated. Activations feature-major ("T" = [feature, batch]).

Mixed layer out = sum_e coeff[:,e] * (x @ W_e) + coeff @ b is computed as
PSUM-accumulated matmuls where the stationary operand is the per-expert
coeff-scaled input X'[k,e,b] = x^T[k,b] * coeff[b,e] and the weights stream
512 wide.

- w0/w1 in HBM as fp8 e3m4 scaled x32 (1/32 folded into S = coeff/32 on-chip,
  w2*32 on host, so outputs are true-scale); w2/gating/activations fp16.
  ~3.4 MB HBM traffic per core (vs 7.2 bf16).
- Weight DMAs are plain [P, N] contiguous copies split across sync/scalar
  HWDGE + gpsimd SWDGE rings, ordered by first use. The gating pack is split
  in three chunks, one leading each ring, so gating can start ~1 us after the
  rings open regardless of which ring wins the packet round-robin.
- Matmuls are column-paired: two accumulation chains into PSUM partition
  halves [0:64] / [64:128] via PE column-group tiling; two k-tiles stream
  concurrently. Halves are summed during the seam; layer 2's on host.
- ELU as elu(x) + 1 = min(exp(x), 1) + relu(x); the -1 offset is folded into
  the next layer's bias on host (colsum of the quantized weights).
- Seam pipeline per 128-col chunk: gpsimd copies the B half out of PSUM, ACT
  does exp+relu, DVE does sum/combine/rescale; the transposed h' is read
  straight from PSUM by the rescale. Expert 4:8 rescale is deferred under the
  next layer's e4:8 matmul stream. Each layer's z k-tiles are emitted right
  after the previous layer's chain so the PE stream stays dense (which also
  flips the HAM clock gate to 2.4 GHz ~4 us into layer 0/1).
"""

import numpy as np
import ml_dtypes

import concourse.tile as tile
from concourse import bacc, mybir
from concourse import bass_utils

F16 = mybir.dt.float16
F8 = mybir.dt.float8e3
F32 = mybir.dt.float32
AF = mybir.ActivationFunctionType
OP = mybir.AluOpType

B, L, FS, H, E = 512, 64, 96, 512, 8
IN = L + FS          # 160
INTER = L + H        # 576
OUT = FS             # 96
NCORES = 8
BL = B // NCORES     # 64 batch rows per core
SC = 32.0            # weight scale folded into S = coeff/SC

_nhf = np.float16
_nf8 = ml_dtypes.float8_e3m4

# gpack column layout (fp16 [128, 784]), split into 3 DMA chunks:
# chunk0 [0:448) on sync, chunk1 [448:656) on scalar, chunk2 [656:784) gpsimd
_GP_ZCT = 0       # zcT: [80, 2, 64] k-tiles (rows 0:80 | 80:160)
_GP_GW0 = 128     # gw0: same 80/80 split
_GP_GB0 = 256     # [1, 64]
_GP_ONES = 320    # [1, 128] ones
_GP_C1 = 448
_GP_GW1 = 448     # [64, 64]
_GP_GW2 = 512     # [64, 8]
_GP_GB1 = 520     # [1, 64]
_GP_GB2 = 584     # [1, 8]
_GP_ID = 592      # [64, 64] identity
_GP_C2 = 656
_GP_INV = 656     # [1, 128] value 1/SC
_GP_COLS = 784


def _build():
    nc = bacc.Bacc("TRN2", target_bir_lowering=False, debug=False,
                   num_devices=NCORES)

    def din(name, shape, dtype):
        return nc.dram_tensor(name, list(shape), dtype,
                              kind="ExternalInput").ap()

    gpk0 = din("gpk0", [128, _GP_C1], F16)
    gpk1 = din("gpk1", [128, _GP_C2 - _GP_C1], F16)
    gpk2 = din("gpk2", [128, _GP_COLS - _GP_C2], F16)
    bcat = din("bcat", [E, 1120], F16)
    w0a = din("w0a", [80, E * 512], F8)
    w0b = din("w0b", [80, E * 512], F8)
    w1lo = din("w1lo", [128, 4 * 4 * 512], F8)
    w1hi = din("w1hi", [128, 4 * 4 * 512], F8)
    w1zp = din("w1zp", [L, E * 512], F8)
    w2hp = din("w2hp", [128, E * 4 * OUT], F16)
    w2zp = din("w2zp", [L, E * OUT], F16)

    # both PSUM halves; host sums rows 0:64 + 64:128
    out_d = nc.dram_tensor("out", [128, OUT], F32, kind="ExternalOutput").ap()

    with tile.TileContext(nc) as tc:
        with (
            tc.tile_pool(name="const", bufs=1) as cpool,
            tc.tile_pool(name="w", bufs=1) as wpool,
            tc.tile_pool(name="x", bufs=1) as xpool,
            tc.tile_pool(name="x12", bufs=2) as x12pool,
            tc.tile_pool(name="act", bufs=2) as apool,
            tc.tile_pool(name="psg", bufs=1, space="PSUM") as psg,
            tc.tile_pool(name="psm", bufs=2, space="PSUM") as psm,
            tc.tile_pool(name="pso", bufs=1, space="PSUM") as pso,
        ):
            gp = cpool.tile([128, _GP_COLS], F16, tag="gp")
            w0t = wpool.tile([80, 2, E, 512], F8, tag="w0")
            w1t = wpool.tile([128, E, 4, 512], F8, tag="w1")
            w1zt = wpool.tile([L, E, 512], F8, tag="w1z")
            w2t = wpool.tile([128, E, 4, OUT], F16, tag="w2")
            w2zt = wpool.tile([L, E, OUT], F16, tag="w2z")
            bc = cpool.tile([E, 1120], F16, tag="bc")

            # ---- DMAs: 3 rings, gpack chunk first on each ----
            nc.sync.dma_start(gp[:, 0:_GP_C1], gpk0[:])
            nc.scalar.dma_start(gp[:, _GP_C1:_GP_C2], gpk1[:])
            nc.gpsimd.dma_start(gp[:, _GP_C2:_GP_COLS], gpk2[:])
            nc.sync.dma_start(
                w0t[:, 0].rearrange("p e o -> p (e o)"), w0a[:])
            nc.scalar.dma_start(
                w0t[:, 1].rearrange("p e o -> p (e o)"), w0b[:])
            nc.gpsimd.dma_start(
                w1t[:, 0:4].rearrange("p e t o -> p (e t o)"), w1lo[:])
            nc.scalar.dma_start(bc[:], bcat[:])
            nc.scalar.dma_start(w1zt[:].rearrange("p e o -> p (e o)"), w1zp[:])
            nc.scalar.dma_start(
                w1t[:, 4:8].rearrange("p e t o -> p (e t o)"), w1hi[:])
            nc.gpsimd.dma_start(
                w2t[:].rearrange("p e t o -> p (e t o)"), w2hp[:])
            nc.gpsimd.dma_start(w2zt[:].rearrange("p e o -> p (e o)"), w2zp[:])

            # gpack views
            zcT0 = gp[0:80, _GP_ZCT:_GP_ZCT + 64]
            zcT1 = gp[0:80, _GP_ZCT + 64:_GP_ZCT + 128]
            zT = gp[0:L, _GP_ZCT:_GP_ZCT + 64]
            gw00 = gp[0:80, _GP_GW0:_GP_GW0 + 64]
            gw01 = gp[0:80, _GP_GW0 + 64:_GP_GW0 + 128]
            gw1v = gp[0:64, _GP_GW1:_GP_GW1 + 64]
            gw2v = gp[0:64, _GP_GW2:_GP_GW2 + E]
            gb0v = gp[0:1, _GP_GB0:_GP_GB0 + 64]
            gb1v = gp[0:1, _GP_GB1:_GP_GB1 + 64]
            gb2v = gp[0:1, _GP_GB2:_GP_GB2 + E]
            identv = gp[0:64, _GP_ID:_GP_ID + 64]
            ones64 = gp[0:1, _GP_ONES:_GP_ONES + BL]
            inv32 = gp[0:1, _GP_INV:_GP_INV + 128]

            # ---- ELU+1: out = min(exp(x), 1) + relu(x); exp on ACT and relu
            # on DVE in parallel (the -1 is folded into the consumer's bias)
            def elu1(dst, src, shape):
                rl = apool.tile(shape, F32, tag="elu_rl", bufs=4)
                ex = apool.tile(shape, F32, tag="elu_ex", bufs=4)
                nc.scalar.activation(ex[:], src, AF.Exp)
                nc.vector.tensor_scalar_max(rl[:], src, 0.0)
                nc.vector.scalar_tensor_tensor(dst, ex[:], 1.0, rl[:],
                                               OP.min, OP.add)

            # ---- gating ----
            g1ps = psg.tile([64, 64], F32, tag="gps", bufs=1)
            nc.tensor.matmul(g1ps[:], gb0v, ones64, start=True, stop=False)
            nc.tensor.matmul(g1ps[:], gw00, zcT0, start=False, stop=False)
            nc.tensor.matmul(g1ps[:], gw01, zcT1, start=False, stop=True)
            g1_t = apool.tile([64, 64], F16, tag="g1")
            elu1(g1_t[:], g1ps[:], [64, 64])

            g2ps = psg.tile([64, 64], F32, tag="gps", bufs=1)
            nc.tensor.matmul(g2ps[:], gb1v, ones64, start=True, stop=False)
            nc.tensor.matmul(g2ps[:], gw1v, g1_t[:], start=False, stop=True)
            g2_t = apool.tile([64, 64], F16, tag="g2")
            elu1(g2_t[:], g2ps[:], [64, 64])

            # logits batch-major [b, e]
            lgps = psg.tile([64, E], F32, tag="gps", bufs=1)
            nc.tensor.matmul(lgps[:], ones64, gb2v, start=True, stop=False)
            nc.tensor.matmul(lgps[:], g2_t[:], gw2v, start=False, stop=True)

            exps_t = apool.tile([64, E], F32, tag="exps")
            se_t = apool.tile([64, 1], F32, tag="se")
            nc.scalar.activation(exps_t[:], lgps[:], AF.Exp, accum_out=se_t[:])
            rec_t = apool.tile([64, 1], F32, tag="rec")
            nc.vector.reciprocal(rec_t[:], se_t[:])
            coeff_t = apool.tile([64, E], F16, tag="coeff")
            nc.vector.tensor_scalar(coeff_t[:], exps_t[:], rec_t[:], None,
                                    OP.mult)

            # coeff transposes on the PE; S[p,e,b] = coeff[b,e]/SC everywhere
            misc = psg.tile([E, 576], F16, tag="misc", bufs=1)
            for e in range(E):
                nc.tensor.matmul(misc[0:1, 64 + 64 * e:128 + 64 * e],
                                 coeff_t[:, e:e + 1], identv,
                                 is_transpose=True, start=True, stop=True)
            nc.tensor.matmul(misc[:, 0:64], coeff_t[:], identv,
                             is_transpose=True, start=True, stop=True)
            rows_t = cpool.tile([1, E, BL], F16, tag="rows")
            nc.vector.tensor_copy(rows_t[:].rearrange("p a b -> p (a b)"),
                                  misc[0:1, 64:576])
            coeffT_t = cpool.tile([E, BL], F16, tag="coeffT")
            nc.vector.tensor_copy(coeffT_t[:], misc[:, 0:64])
            S_ps = pso.tile([128, E, BL], F32, tag="S")
            for e in range(E):
                nc.tensor.matmul(S_ps[:, e, :], inv32,
                                 rows_t[0:1, e, :], start=(e == 0),
                                 stop=(e == E - 1))

            # layer-0 x' straight off PSUM; experts 0:2 first so layer 0's
            # first pair can fire immediately
            x0t = xpool.tile([80, 2, E, BL], F16, tag="x0")

            def x0scale(es):
                n = es.stop - es.start
                nc.vector.tensor_tensor(
                    x0t[:, 0, es, :],
                    zcT0.unsqueeze(1).broadcast_to((80, n, BL)),
                    S_ps[0:80, es, :], OP.mult)
                nc.vector.tensor_tensor(
                    x0t[:, 1, es, :],
                    zcT1.unsqueeze(1).broadcast_to((80, n, BL)),
                    S_ps[0:80, es, :], OP.mult)

            x0scale(slice(0, 2))
            x0scale(slice(2, 8))

            S_t = cpool.tile([128, E, BL], F16, tag="S")
            nc.vector.tensor_copy(S_t[:], S_ps[:])

            def xscale_psum(xt, hTp, t, es):
                # in0 from PSUM (transposed h'), in1 fp16 S from SBUF
                n = es.stop - es.start
                nc.vector.tensor_tensor(
                    xt[:, t, es, :],
                    hTp[:, t, :].unsqueeze(1).broadcast_to((128, n, BL)),
                    S_t[:, es, :], OP.mult)

            def xscale_z(xt):
                nc.gpsimd.tensor_tensor(
                    xt[0:L, 4, :, :],
                    zT.unsqueeze(1).broadcast_to((L, E, BL)),
                    S_t[0:L, :, :], OP.mult)

            x1t = x12pool.tile([128, 5, E, BL], F16, tag="x12")
            xscale_z(x1t)
            x2t = x12pool.tile([128, 5, E, BL], F16, tag="x12")

            LO, HI = slice(0, 4), slice(4, 8)

            def seam(l_ps, xt_next, emit_chunk, emit_tail):
                """halves-sum + ELU+1 + transpose + rescale per 128-col chunk;
                emit_chunk(m) queues the next layer's e0:4 matmuls for k-tile
                m; emit_tail() the e4:8 ones (their rescale is deferred)."""
                hb = apool.tile([64, 512], F16, tag="hb")
                hTp = psm.tile([128, 4, BL], F16, tag="hTp", bufs=2)
                for h2 in range(2):
                    sl = slice(256 * h2, 256 * (h2 + 1))
                    su = apool.tile([64, 256], F32, tag="sm_su", bufs=2)
                    bcop = apool.tile([64, 256], F32, tag="sm_bc", bufs=2)
                    rl = apool.tile([64, 256], F32, tag="sm_rl", bufs=2)
                    ex = apool.tile([64, 256], F32, tag="sm_ex", bufs=2)
                    nc.scalar.activation(bcop[:], l_ps[64:128, sl], AF.Copy)
                    nc.vector.tensor_tensor(su[:], l_ps[0:64, sl],
                                            bcop[:], OP.add)
                    nc.scalar.activation(ex[:], su[:], AF.Exp)
                    nc.vector.tensor_scalar_max(rl[:], su[:], 0.0)
                    nc.vector.scalar_tensor_tensor(hb[:, sl], ex[:], 1.0,
                                                   rl[:], OP.min, OP.add)
                    for m in (2 * h2, 2 * h2 + 1):
                        msl = slice(128 * m, 128 * (m + 1))
                        nc.tensor.matmul(hTp[:, m, :], hb[:, msl], identv,
                                         is_transpose=True, start=True,
                                         stop=True)
                        xscale_psum(xt_next, hTp, m, LO)
                        emit_chunk(m)
                for m in range(4):
                    xscale_psum(xt_next, hTp, m, HI)
                emit_tail()

            # ---- layer 0 (paired: t=0/K=128 -> rows 0:64, t=1/K=32) ----
            l0ps = psm.tile([128, H], F32, tag="lps")
            for e in range(E):
                nc.tensor.matmul(l0ps[0:64, :], x0t[:, 0, e, :],
                                 w0t[:, 0, e, :],
                                 start=(e == 0), stop=False)
                nc.tensor.matmul(l0ps[64:128, :], x0t[:, 1, e, :],
                                 w0t[:, 1, e, :],
                                 start=(e == 0), stop=(e == E - 1),
                                 skip_group_check=True)
            nc.tensor.matmul(l0ps[0:64, :], coeffT_t[:], bc[:, 0:512],
                             start=False, stop=True)

            # ---- layer 1 (paired: even e -> rows 0:64, odd e -> 64:128);
            # z k-tiles first to cover the seam's ELU latency ----
            l1ps = psm.tile([128, H], F32, tag="lps")

            def l1_mm(e, t, first, last):
                half = slice(0, 64) if e % 2 == 0 else slice(64, 128)
                if t < 4:
                    lhs, rhs = x1t[:, t, e, :], w1t[:, e, t, :]
                else:
                    lhs, rhs = x1t[0:L, 4, e, :], w1zt[:, e, :]
                nc.tensor.matmul(l1ps[half, :], lhs, rhs,
                                 start=first, stop=last,
                                 skip_group_check=(e % 2 == 1))

            for e in range(E):
                l1_mm(e, 4, (e <= 1), False)   # z k-tiles early start chains

            def l1_chunk(m):
                for eb in (0, 2):
                    l1_mm(eb, m, False, False)
                    l1_mm(eb + 1, m, False, False)

            def l1_tail():
                xscale_z(x2t)
                for eb in (4, 6):
                    for t in range(4):
                        l1_mm(eb, t, False, False)
                        l1_mm(eb + 1, t, False, (eb == 6 and t == 3))
                nc.tensor.matmul(l1ps[0:64, :], coeffT_t[:], bc[:, 512:1024],
                                 start=False, stop=True)

            seam(l0ps, x1t, l1_chunk, l1_tail)

            # ---- layer 2 (paired, N=96; halves summed on host) ----
            l2ps = pso.tile([128, 512], F32, tag="ops")

            def l2_mm(e, t, first, last):
                half = slice(0, 64) if e % 2 == 0 else slice(64, 128)
                if t < 4:
                    lhs, rhs = x2t[:, t, e, :], w2t[:, e, t, :]
                else:
                    lhs, rhs = x2t[0:L, 4, e, :], w2zt[:, e, :]
                nc.tensor.matmul(l2ps[half, 0:OUT], lhs, rhs,
                                 start=first, stop=last,
                                 skip_group_check=(e % 2 == 1))

            def l2_pre():
                # z k-tiles right after layer 1's chain
                for e in range(E):
                    l2_mm(e, 4, (e <= 1), False)

            def l2_chunk(m):
                if m == 0:
                    l2_pre()
                for eb in (0, 2):
                    l2_mm(eb, m, False, False)
                    l2_mm(eb + 1, m, False, False)

            def l2_tail():
                for eb in (4, 6):
                    for t in range(4):
                        l2_mm(eb, t, False, False)
                        l2_mm(eb + 1, t, False, (eb == 6 and t == 3))
                nc.tensor.matmul(l2ps[0:64, 0:OUT], coeffT_t[:],
                                 bc[:, 1024:1120], start=False, stop=True)

            seam(l1ps, x2t, l2_chunk, l2_tail)

            out_t = apool.tile([128, OUT], F32, tag="out_sb")
            nc.vector.tensor_copy(out_t[:], l2ps[:, 0:OUT])
            nc.sync.dma_start(out_d[:], out_t[:])

    nc.compile()
    return nc


_NC_CACHE = None


def _get_nc():
    global _NC_CACHE
    if _NC_CACHE is None:
        _NC_CACHE = _build()
    return _NC_CACHE


def _host_prep(z, c, gw0, gb0, gw1, gb1, gw2, gb2, w0, b0, w1, b1, w2, b2):
    hf = lambda a: np.ascontiguousarray(a).astype(_nhf)
    f8 = lambda a: np.ascontiguousarray(a * SC).astype(_nf8)

    gw1 = np.asarray(gw1)
    gw2 = np.asarray(gw2)
    # elu outputs carry a +1 offset; correct in the consumer's bias.
    gb1c = np.asarray(gb1) - gw1.sum(axis=0)
    gb2c = np.asarray(gb2) - gw2.sum(axis=0)

    gp_base = np.zeros((128, _GP_COLS), dtype=np.float32)
    gw0 = np.asarray(gw0)
    gp_base[0:80, _GP_GW0:_GP_GW0 + 64] = gw0[0:80]
    gp_base[0:80, _GP_GW0 + 64:_GP_GW0 + 128] = gw0[80:160]
    gp_base[0:64, _GP_GW1:_GP_GW1 + 64] = gw1
    gp_base[0:64, _GP_GW2:_GP_GW2 + E] = gw2
    gp_base[0, _GP_GB0:_GP_GB0 + 64] = gb0
    gp_base[0, _GP_GB1:_GP_GB1 + 64] = gb1c
    gp_base[0, _GP_GB2:_GP_GB2 + E] = gb2c
    gp_base[0:64, _GP_ID:_GP_ID + 64] = np.eye(64, dtype=np.float32)
    gp_base[0, _GP_ONES:_GP_ONES + 128] = 1.0
    gp_base[0, _GP_INV:_GP_INV + 128] = 1.0 / SC

    w0 = np.asarray(w0)
    w1 = np.asarray(w1)
    w2 = np.asarray(w2)
    # w0 k-tiles of 80 rows each
    w0ka = w0[:, 0:80, :].transpose(1, 0, 2)       # [80, E, H]
    w0kb = w0[:, 80:160, :].transpose(1, 0, 2)     # [80, E, H]
    # w1 h-part [128, E, 4, 512] (quantized); z-part [64, E, 512]
    w1h = w1[:, L:INTER, :].reshape(E, 4, 128, H).transpose(2, 0, 1, 3)
    w1z = w1[:, 0:L, :].transpose(1, 0, 2)
    w1h8 = f8(w1h)
    # w2 x SC in fp16 (x2t carries 1/SC)
    w2h = (w2[:, L:INTER, :] * SC).reshape(E, 4, 128, OUT).transpose(2, 0, 1, 3)
    w2z = (w2[:, 0:L, :] * SC).transpose(1, 0, 2)
    w2hh = hf(w2h)

    # h offsets: +1 on the h-part inputs of layers 1/2 -> bias correction
    # using the exact (de)quantized weights the kernel will multiply by.
    w1h_deq = w1h8.astype(np.float32) / SC            # [128, E, 4, H]
    b1c = np.asarray(b1) - w1h_deq.sum(axis=(0, 2))
    w2h_deq = w2hh.astype(np.float32) / SC            # [128, E, 4, OUT]
    b2c = np.asarray(b2) - w2h_deq.sum(axis=(0, 2))

    shared = {
        "w0a": f8(w0ka).reshape(80, E * 512),
        "w0b": f8(w0kb).reshape(80, E * 512),
        "w1lo": np.ascontiguousarray(w1h8[:, 0:4]).reshape(128, 4 * 4 * 512),
        "w1hi": np.ascontiguousarray(w1h8[:, 4:8]).reshape(128, 4 * 4 * 512),
        "w1zp": f8(w1z).reshape(L, E * 512),
        "w2hp": w2hh.reshape(128, E * 4 * OUT),
        "w2zp": hf(w2z).reshape(L, E * OUT),
        "bcat": hf(np.concatenate([b0, b1c, b2c], axis=1)),
    }
    zc = np.concatenate([np.asarray(z), np.asarray(c)], axis=1)  # [B, IN]
    in_maps = []
    for i in range(NCORES):
        gpi = gp_base.copy()
        zcT = zc[i * BL:(i + 1) * BL, :].T  # [IN, 64]
        gpi[0:80, _GP_ZCT:_GP_ZCT + 64] = zcT[0:80]
        gpi[0:80, _GP_ZCT + 64:_GP_ZCT + 128] = zcT[80:160]
        m = dict(shared)
        m["gpack"] = hf(gpi)
        m["gpk0"] = hf(gpi[:, 0:_GP_C1])
        m["gpk1"] = hf(gpi[:, _GP_C1:_GP_C2])
        m["gpk2"] = hf(gpi[:, _GP_C2:_GP_COLS])
        del m["gpack"]
        in_maps.append(m)
    return in_maps


def kernel(**inputs):
    nc = _get_nc()
    in_maps = _host_prep(**inputs)
    res = bass_utils.run_bass_kernel_spmd(nc, in_maps,
                                          core_ids=list(range(NCORES)))
    return np.concatenate(
        [r["out"][0:BL] + r["out"][BL:128] for r in res.results], axis=0)
